# revision 90
# baseline (speedup 1.0000x reference)
"""Trainium2 Bass kernel for the MemoryReader (retrieval-knn) module.

Math (per batch b):
    a[m]     = sum_ck mk[ck, m]^2
    logits   = (2 * mk^T qk - a) / sqrt(CK)        # [THW, NQ]
    aff      = softmax(logits, axis=THW)
    out      = mv @ aff                            # [CV, NQ]

Shapes: B=4, CK=64, T=8, H=30, W=54 (THW=12960, NQ=1620), CV=512.

Sharding: 8 cores = (B=4) x (THW halves of 6480, padded to 6528).  Each
core computes UNNORMALIZED partial readout acc[CV, NQ] and partial
denominator den[128, NQ] over its half of the memory tokens; the host
sums the two halves and divides (flash-attention-style split; safe with
no max subtraction since logits are in [-27, 4]).

mv (6.6 MB/core in bf16) is loaded ONCE into SBUF and reused by all
four query blocks, so the kernel is PE-bound at the full f32r/bf16
matmul rate (~169 ns per 406-wide matmul).

Per query block the CV=512 output channels are split 3+1 into two
phases over the 51 m-tiles:
  phase 1: score -> exp (bf16, persisted in SBUF) -> den add
           + acc matmuls for cv tiles 0..2, with the accs trailing the
           scores by D tiles (explicit software pipelining)
  phase 2: acc matmuls for cv tile 3 reading the persisted exp
           (pure back-to-back matmul stream; PSUM drains overlap)
The 3+1 split frees a PSUM bank so the score pool gets 5 buffers --
enough pipeline runway to hide the score->exp->acc semaphore latency
that otherwise stalls the PE every few m-tiles (PSUM has 8 banks:
3 acc + 5 score; the phase-2 accumulator borrows an idle score slot).

Other tricks:
  - squared-norm folded into the score matmul (lhsT' = [mk; mk^2],
    rhs' = [qk; -0.5], logits = 0.25 * psum via the ACT scale).
  - THW padded 6480 -> 6528 with mkq pad columns [0; 10] so padded
    tokens get logits = -80 -> exp ~ 0 (no den/acc pollution).
  - readout in bf16, scores in f32r (full PE rate for both; f32r
    scores buy fp8 error budget); accumulation in f32 PSUM,
    denominators in f32 on DVE.
  - the last 16 m-tiles of the three wide query blocks run in fp8 e4m3
    with DoubleRow matmuls (256-token contraction at 0.5 cycles/row,
    4x the bf16 rate); their scores/exps are interleaved into the
    phase-2 stream so the Act engine keeps up.
  - all exps carry a +2.0 bias (exp(logits+2), cancels in the host
    normalization): it shifts the softmax mass out of fp8-e4m3's
    subnormal/underflow zone, cutting fp8 quantization noise ~30%
    while the max logit (+4.0) stays under e4m3's 448 cap.  Total rel
    err 1.85e-2 vs the 2e-2 budget (matches a numpy model to 5
    digits).
  - mv host-pre-swizzled to the SBUF layout [128, 51*512] so the whole
    preload is 13 large contiguous-per-partition DMAs, interleaved with
    mkq chunks in consumption order (each DMA costs ~625 ns of
    serialized HWDGE issue, so fewer/larger is better).
  - query blocks [440, 440, 440, 300]: the narrow last block shortens
    the end-of-kernel drain tail.
  - dummy warmup matmuls during the initial DMA fill so the PE p-state
    ramp (0.65/1.2 GHz for the first ~3 us of activity) is spent on
    junk work instead of real scores.
"""

import os
import sys

import numpy as np

for _p in ("/opt/trn_rl_repo",):
    if _p not in sys.path and os.path.isdir(_p):
        sys.path.insert(0, _p)

B, CK, T, H, W = 4, 64, 8, 30, 54
CV = 512
THW = T * H * W          # 12960
NQ = H * W               # 1620
MH = THW // 2            # 6480  memory tokens per core
P = 128
MT = (MH + P - 1) // P   # 51 m-tiles
MHP = MT * P             # 6528 padded
QBLKS = [(0, 440), (440, 440), (880, 440), (1320, 300)]  # narrow last block -> short tail
QW = 440                 # ex_all column stride per m-tile
NCV = CV // P            # 4 output-channel tiles
N_WARM = 13              # dummy PE warmup matmuls
NPAIR = 8                # fp8 DoubleRow m-tile pairs (blocks 0-2; last block bf16)
FP8_START = MT - 2 * NPAIR  # first fp8 m-tile (35)
EXP_BIAS = 2.0           # exp(logits + 2): shifts the softmax mass out of
                         # fp8-e4m3's subnormal/underflow zone (cancels in
                         # the host-side acc/den normalization); max logit
                         # ~4 -> e^6 = 403 < 448 (e4m3 max), no clipping

_PROGRAM = None


def _build_program():
    import concourse.mybir as mybir
    import concourse.tile as tile
    from concourse import bacc

    f32 = mybir.dt.float32
    f32r = mybir.dt.float32r
    bf16 = mybir.dt.bfloat16
    f8 = mybir.dt.float8e4
    DR = mybir.MatmulPerfMode.DoubleRow
    Exp = mybir.ActivationFunctionType.Exp

    nc = bacc.Bacc(
        "TRN2",
        target_bir_lowering=False,
        debug=False,
        enable_asserts=False,
        num_devices=8,
    )

    mkq = nc.dram_tensor("mkq", [P, MHP], f32r, kind="ExternalInput").ap()
    qkc = nc.dram_tensor("qkc", [P, NQ], f32r, kind="ExternalInput").ap()
    mvb = nc.dram_tensor("mvb", [P, MT * CV], bf16, kind="ExternalInput").ap()
    mv8 = nc.dram_tensor("mv8", [P, NPAIR * 2 * CV], f8, kind="ExternalInput").ap()
    acc_o = nc.dram_tensor("acc", [CV, NQ], f32, kind="ExternalOutput").ap()
    den_o = nc.dram_tensor("den", [P, NQ], f32, kind="ExternalOutput").ap()

    # 4-m-tile DMA groups (last has 3).  Fewer, larger DMAs: each DMA costs
    # ~625 ns of serialized HWDGE issue, so small chunks delay later ones.
    GROUPS = []
    m0 = 0
    while m0 < MT:
        m1 = min(m0 + 4, MT)
        GROUPS.append(((m0 * P, m1 * P), (m0 * CV, m1 * CV)))
        m0 = m1

    with tile.TileContext(nc) as tc:
        with (
            tc.tile_pool(name="const", bufs=1) as cpool,
            tc.tile_pool(name="dens", bufs=1) as dpool,
            tc.tile_pool(name="outp", bufs=6) as opool,
            tc.tile_pool(name="score_ps", bufs=5, space="PSUM") as spspool,
            tc.tile_pool(name="acc_ps", bufs=1, space="PSUM") as apspool,
        ):
            qkc_sb = cpool.tile([P, NQ], f32r, tag="qkc", name="qkc")
            mkq_sb = cpool.tile([P, MHP], f32r, tag="mkq", name="mkq")
            mv_sb = cpool.tile([P, MT * CV], bf16, tag="mv", name="mv")
            mv8_sb = cpool.tile([P, NPAIR, 2, CV], f8, tag="mv8", name="mv8")
            ex_all = cpool.tile([P, MT * QW], bf16, tag="ex", name="ex")
            ex8 = cpool.tile([P, NPAIR, 2, QW], f8, tag="ex8", name="ex8")
            warm = cpool.tile([P, 256], f32, tag="warm", name="warm")
            nc.vector.memset(warm[:], 0.0)
            ebias = cpool.tile([P, 1], f32, tag="ebias", name="ebias")
            nc.vector.memset(ebias[:], EXP_BIAS)

            # DMA order = consumption order: qkc block 0 first so the first
            # score can issue ASAP, then interleaved mkq/mv groups.
            nq0 = QBLKS[0][1]
            nc.sync.dma_start(out=qkc_sb[:, :nq0], in_=qkc[:, :nq0])
            for gi, ((ka, kb), (va, vb)) in enumerate(GROUPS):
                nc.sync.dma_start(out=mkq_sb[:, ka:kb], in_=mkq[:, ka:kb])
                nc.sync.dma_start(out=mv_sb[:, va:vb], in_=mvb[:, va:vb])
                if gi == 2:
                    nc.sync.dma_start(out=qkc_sb[:, nq0:], in_=qkc[:, nq0:])
            nc.sync.dma_start(out=mv8_sb[:, :, :, :], in_=mv8[:, :])

            # PE p-state warmup on zeros while the first DMAs land.
            for _ in range(N_WARM):
                w_ps = spspool.tile([P, QW], f32, tag="score", name="score")
                nc.tensor.matmul(
                    w_ps[:2, :256],
                    lhsT=warm[:, :2].bitcast(f32r),
                    rhs=warm[:].bitcast(f32r),
                    start=True,
                    stop=True,
                )

            for qi, (q0, nq) in enumerate(QBLKS):
                # fp8 pairs on the three wide blocks only: the narrow last
                # block's DR savings are eaten by its seam/tail costs, and
                # keeping it bf16 buys error margin for a 4th pair here.
                npb = NPAIR if qi < 3 else 0
                fp8s = MT - 2 * npb
                accs = [
                    apspool.tile([P, nq], f32, tag=f"acc{c}", name=f"acc{c}")
                    for c in range(3)
                ]
                den = dpool.tile([P, nq], f32, tag=f"den{qi}", name=f"den{qi}")
                nc.vector.memset(den[:], 0.0)

                # phase 1: scores + exp + den + acc for cv tiles 0..2.
                # The acc matmuls trail the scores by D tiles (explicit
                # software pipelining) so each score is emitted -- and thus
                # scheduled, PE is in-order -- well before its exp's
                # consumers, hiding the score->exp->acc semaphore lap.
                D = 4
                for mi in range(fp8s + D):
                    if mi < fp8s:
                        score = spspool.tile([P, nq], f32, tag="score", name="score")
                        nc.tensor.matmul(
                            score[:],
                            lhsT=mkq_sb[:, mi * P : (mi + 1) * P],
                            rhs=qkc_sb[:, q0 : q0 + nq],
                            start=True,
                            stop=True,
                        )
                        ex = ex_all[:, mi * QW : mi * QW + nq]
                        nc.scalar.activation(ex, score[:], Exp, bias=ebias[:], scale=0.25)
                        nc.vector.tensor_add(den[:], den[:], ex)
                    if mi >= D:
                        md = mi - D
                        exd = ex_all[:, md * QW : md * QW + nq]
                        for c in range(3):
                            nc.tensor.matmul(
                                accs[c][:],
                                lhsT=mv_sb[:, md * CV + c * P : md * CV + (c + 1) * P],
                                rhs=exd,
                                start=(md == 0),
                                stop=(npb == 0 and md == fp8s - 1),
                            )
                # Allocate the phase-2 accumulator's score-pool slot now:
                # its WAR lands on a long-finished exp instead of an
                # in-flight fp8 exp from phase 1b.
                acc3 = spspool.tile([P, nq], f32, tag="score", name="acc3")

                # phase 1b/2a interleaved: the fp8 pair scores ride one
                # per two bf16 acc3 tiles so score production is throttled
                # to the Act engine's exp rate (no score-pool pile-up); the
                # DoubleRow matmuls come after PRERUN tiles of exp8 runway.
                PRERUN = 34 if npb else 2
                for mi in range(PRERUN):
                    if mi % 2 == 0 and mi // 2 < 2 * npb:
                        k = mi // 2
                        mj = fp8s + k
                        score = spspool.tile([P, nq], f32, tag="score", name="score")
                        nc.tensor.matmul(
                            score[:],
                            lhsT=mkq_sb[:, mj * P : (mj + 1) * P],
                            rhs=qkc_sb[:, q0 : q0 + nq],
                            start=True,
                            stop=True,
                        )
                        e8 = ex8[:, k // 2, k % 2, :nq]
                        nc.scalar.activation(e8, score[:], Exp, bias=ebias[:], scale=0.25)
                        nc.vector.tensor_add(den[:], den[:], e8)
                    nc.tensor.matmul(
                        acc3[:],
                        lhsT=mv_sb[:, mi * CV + 3 * P : mi * CV + 4 * P],
                        rhs=ex_all[:, mi * QW : mi * QW + nq],
                        start=(mi == 0),
                        stop=False,
                    )
                # fp8 DoubleRow matmuls: each contracts 256 memory tokens at
                # 0.5 cycles/row -- 4x the bf16 rate.  Completes accs[0..2]
                # so their drains overlap the rest of phase 2.
                for pr in range(npb):
                    for c in range(3):
                        nc.tensor.matmul(
                            accs[c][:],
                            lhsT=mv8_sb[:, pr, :, c * P : (c + 1) * P],
                            rhs=ex8[:, pr, :, :nq],
                            start=False,
                            stop=(pr == npb - 1),
                            perf_mode=DR,
                        )
                    nc.tensor.matmul(
                        acc3[:],
                        lhsT=mv8_sb[:, pr, :, 3 * P : 4 * P],
                        rhs=ex8[:, pr, :, :nq],
                        start=False,
                        stop=False,
                        perf_mode=DR,
                    )
                for mi in range(PRERUN, fp8s):
                    nc.tensor.matmul(
                        acc3[:],
                        lhsT=mv_sb[:, mi * CV + 3 * P : mi * CV + 4 * P],
                        rhs=ex_all[:, mi * QW : mi * QW + nq],
                        start=False,
                        stop=(mi == fp8s - 1),
                    )
                for c in range(3):
                    o = opool.tile([P, nq], f32, tag="out", name="out")
                    # Alternate drain engines (DVE / Act) so copies overlap.
                    if c % 2 == 0:
                        nc.vector.tensor_copy(o[:], accs[c][:])
                    else:
                        nc.scalar.copy(o[:], accs[c][:])
                    nc.sync.dma_start(
                        out=acc_o[c * P : (c + 1) * P, q0 : q0 + nq], in_=o[:]
                    )
                nc.sync.dma_start(out=den_o[:, q0 : q0 + nq], in_=den[:])
                o3 = opool.tile([P, nq], f32, tag="out", name="out")
                nc.vector.tensor_copy(o3[:], acc3[:])
                nc.sync.dma_start(out=acc_o[3 * P : 4 * P, q0 : q0 + nq], in_=o3[:])

    nc.compile()
    return nc


def _get_program():
    global _PROGRAM
    if _PROGRAM is None:
        _PROGRAM = _build_program()
    return _PROGRAM


def _make_in_maps(mk, qk, mv):
    import ml_dtypes

    mkf = np.ascontiguousarray(mk.reshape(B, CK, THW), dtype=np.float32)
    qkf = np.ascontiguousarray(qk.reshape(B, CK, NQ), dtype=np.float32)
    mvf = mv.reshape(B, CV, THW)

    in_maps = []
    for b in range(B):
        qkc_b = np.ascontiguousarray(
            np.concatenate([qkf[b], np.full((CK, NQ), -0.5, np.float32)], axis=0)
        )  # [128, NQ] f32
        for h in range(2):
            sl = slice(h * MH, (h + 1) * MH)
            mkh = mkf[b][:, sl]
            mkq_b = np.zeros((P, MHP), np.float32)
            mkq_b[:CK, :MH] = mkh
            mkq_b[CK:, :MH] = mkh * mkh
            mkq_b[CK:, MH:] = 10.0  # pad tokens: logits=-80 -> exp ~ 0
            mvt = np.zeros((MHP, CV), np.float32)
            mvt[:MH] = mvf[b][:, sl].T
            mv_sw = np.ascontiguousarray(
                mvt.reshape(MT, P, CV).transpose(1, 0, 2).reshape(P, MT * CV)
            ).astype(ml_dtypes.bfloat16)
            mv8_b = np.ascontiguousarray(
                mvt.reshape(MT, P, CV)[FP8_START:]
                .reshape(NPAIR, 2, P, CV)
                .transpose(2, 0, 1, 3)
                .reshape(P, NPAIR * 2 * CV)
            ).astype(ml_dtypes.float8_e4m3fn)
            in_maps.append({"mkq": mkq_b, "qkc": qkc_b, "mvb": mv_sw, "mv8": mv8_b})
    return in_maps


def kernel(mk, qk, mv, _trace=False, _results_out=None):
    from concourse import bass_utils

    nc = _get_program()
    in_maps = _make_in_maps(np.asarray(mk), np.asarray(qk), np.asarray(mv))
    res = bass_utils.run_bass_kernel_spmd(
        nc, in_maps, core_ids=list(range(8)), trace=_trace
    )
    if _results_out is not None:
        _results_out.append(res)

    full = np.empty((B, CV, NQ), dtype=np.float32)
    for b in range(B):
        acc = res.results[2 * b]["acc"] + res.results[2 * b + 1]["acc"]
        den = (
            res.results[2 * b]["den"].sum(axis=0)
            + res.results[2 * b + 1]["den"].sum(axis=0)
        )
        full[b] = acc / den[None, :]
    return full.reshape(B, CV, H, W)
